# revision 44
# baseline (speedup 1.0000x reference)
"""Distributed multi-head attention kernel for 8 TRN2 NeuronCores.

Reference computation:
    x:[2,2048,1024] -> qkv -> 16-head attention -> proj -> [2,2048,1024]

Sharding: tensor-parallel over heads (2 heads/core) for qkv + attention,
then an AllToAll switches to token sharding (512 tokens/core) for the
projection, so no AllReduce is needed and each core emits only its own
output shard.

Device program structure (per core; PE computes out = lhsT.T @ rhs with
contraction on the partition axis):
  - x^T arrives c-major ([128, 8, 4096] bf16); q/k/v computed d-major
    with c-outer accumulation so matmuls start as x chunks land.
  - V^T for the PV matmul is produced by DMA xbar transposes (frees PE).
  - attention: S^T = K.T @ Q (2 heads row-packed, K=64 each), exp on ACT
    with the 1/8 scale folded in (no max subtraction needed: score std
    ~0.33), PV with lhsT=[1|V^T] so PSUM row 0 accumulates the softmax
    denominator Z for free. QK runs one chunk ahead of PV so PE never
    stalls on the ACT exp.
  - batch 1's qkv matmuls are interleaved into batch 0's (ACT-bound)
    attention loop to use PE slack.
  - AllToAll (1MB bf16) -> token-sharded proj with full weights, bias
    via ACT Identity.
"""

import sys

sys.path.insert(0, "/opt/trn_rl_repo")

import numpy as np
import ml_dtypes

from concourse import bass, bacc, mybir, tile
from concourse.bass_utils import run_bass_kernel_spmd

BF16 = mybir.dt.bfloat16
F32 = mybir.dt.float32
AF = mybir.ActivationFunctionType
ALU = mybir.AluOpType

N_CORES = 8
B, N, C = 2, 2048, 1024
H = 16  # total heads
D = 64  # head dim
T = B * N  # 4096 flattened tokens
TPC = T // N_CORES  # tokens per core = 512
CCH = C // 128  # contraction chunks = 8
SCALE = 1.0 / np.sqrt(D)  # 0.125


def build_bass() -> bass.Bass:
    nc = bacc.Bacc(None, target_bir_lowering=False)

    # ---- DRAM parameters (per-core shards, host-prepared layouts) ----
    xt_d = nc.declare_dram_parameter("xt", [128, CCH, T], BF16, isOutput=False)
    wqkv_d = nc.declare_dram_parameter("wqkv", [128, CCH, 384], BF16, isOutput=False)
    bqkv_d = nc.declare_dram_parameter("bqkv", [128, 3], F32, isOutput=False)
    pwt_d = nc.declare_dram_parameter("pwt", [128, CCH, C], BF16, isOutput=False)
    pb_d = nc.declare_dram_parameter("pb", [128, CCH], F32, isOutput=False)
    out_d = nc.declare_dram_parameter("out", [128, CCH, TPC], F32, isOutput=True)

    with tile.TileContext(nc) as tc:
        with (
            tc.tile_pool(name="const", bufs=1) as const,
            tc.tile_pool(name="weights", bufs=1) as wpool,
            tc.tile_pool(name="acts", bufs=1) as apool,
            tc.tile_pool(name="vt_tmp", bufs=3) as vt_tmp_pool,
            tc.tile_pool(name="dram", bufs=1, space="DRAM") as dram,
        ):
            # ---- resident SBUF tensors ----
            xt = wpool.tile([128, CCH, T], BF16)
            wqkv = wpool.tile([128, CCH, 384], BF16)
            pwt = wpool.tile([128, CCH, C], BF16)
            pb = const.tile([128, CCH], F32)
            bqkv = const.tile([128, 3], F32)

            qkv_sb = apool.tile([128, 3, T], BF16)  # j-major q/k/v
            # PV stationary: per t2-chunk, per head: [1 | V_h^T]
            vt1 = apool.tile([128, T // 128, 2, 65], BF16)
            a2a_sb = apool.tile([128, CCH, TPC], BF16)
            out_sb = apool.tile([128, CCH, TPC], F32)

            a2a_in = dram.tile([N_CORES * 128, TPC], BF16)
            a2a_out = dram.tile([N_CORES * 128, TPC], BF16)
            warm_in = dram.tile([N_CORES, 16], BF16)
            warm_out = dram.tile([N_CORES, 16], BF16)

            # ---- load inputs (wqkv then x chunks; low-priority via scalar q)
            nc.sync.dma_start(out=wqkv[:], in_=wqkv_d[:])
            for c in range(CCH):
                nc.sync.dma_start(out=xt[:, c, :], in_=xt_d[:, c, :])
            nc.scalar.dma_start(out=bqkv[:], in_=bqkv_d[:])
            nc.scalar.dma_start(out=pb[:], in_=pb_d[:])
            nc.sync.dma_start(out=pwt[:], in_=pwt_d[:])
            nc.vector.memset(vt1[:, :, :, 0:1], 1.0)  # Z ones columns

            # tiny dummy AllToAll: warms the collective path so the real
            # A2A's fixed start latency overlaps compute instead
            nc.gpsimd.collective_compute(
                "AllToAll",
                ALU.bypass,
                replica_groups=[list(range(N_CORES))],
                ins=[warm_in[:].opt()],
                outs=[warm_out[:].opt()],
            )

            def finish_qkv_tile(ps, j, t):
                nc.vector.tensor_scalar_add(
                    qkv_sb[:, j, t * 512 : (t + 1) * 512],
                    ps[:],
                    bqkv[:, j : j + 1],
                )
                if j == 2:  # v tile done -> transpose its 4 t2-chunks
                    for ch in range(4 * t, 4 * t + 4):
                        # xbar transpose needs a contiguous dst; then a DVE
                        # copy splits it into the per-head [1|V] layout
                        vtmp = vt_tmp_pool.tile(
                            [128, 2, 64], BF16, tag="vtmp", name="vtmp"
                        )
                        nc.sync.dma_start_transpose(
                            out=vtmp[:, :, :],
                            in_=qkv_sb[:, 2, ch * 128 : (ch + 1) * 128],
                        )
                        nc.vector.tensor_copy(vt1[:, ch, :, 1:65], vtmp[:, :, :])

            def qkv_bulk(qp, js, ts):
                """c-outer qkv tiles for (j, t) pairs; len(js)*len(ts) <= 8
                live PSUM accumulators. Matmuls start as soon as each x
                chunk's DMA lands."""
                pss = {
                    (j, t): qp.tile([128, 512], F32, tag="qps", name="qps")
                    for j in js
                    for t in ts
                }
                for c in range(CCH):
                    for j in js:
                        for t in ts:
                            nc.tensor.matmul(
                                pss[(j, t)][:],
                                wqkv[:, c, j * 128 : (j + 1) * 128],
                                xt[:, c, t * 512 : (t + 1) * 512],
                                start=(c == 0),
                                stop=(c == CCH - 1),
                                skip_group_check=True,
                            )
                for j in js:
                    for t in ts:
                        finish_qkv_tile(pss[(j, t)], j, t)

            def qkv_interleaved_gen(qip, js, ts):
                """Yield-per-matmul qkv tiles (c-inner, single-bank) for
                interleaving into the attention loop's PE slack."""
                for j in js:
                    for t in ts:
                        ps = qip.tile([128, 512], F32, tag="qip", name="qip")
                        for c in range(CCH):
                            nc.tensor.matmul(
                                ps[:],
                                wqkv[:, c, j * 128 : (j + 1) * 128],
                                xt[:, c, t * 512 : (t + 1) * 512],
                                start=(c == 0),
                                stop=(c == CCH - 1),
                                skip_group_check=True,
                            )
                            yield
                        finish_qkv_tile(ps, j, t)
                        yield
                while True:
                    yield

            def attention_batch(b, sp, up, np_pool, ptp, zp, extra=None):
                """Attention for batch b; flat chunk loop, QK one chunk
                ahead of PV across span boundaries."""
                nchunks = N // 128
                nspans = N // 512
                total = nspans * nchunks
                s_tiles = {}
                u_tiles = {}

                def qk(i):
                    span, ch = divmod(i, nchunks)
                    t1 = b * N + span * 512
                    t2 = b * N + ch * 128
                    s = sp.tile([128, 1024], F32, tag="s", name="s")
                    s_tiles[i] = s
                    nc.tensor.matmul(
                        s[:, 0:512],
                        qkv_sb[0:64, 1, t2 : t2 + 128],
                        qkv_sb[0:64, 0, t1 : t1 + 512],
                        start=True,
                        stop=True,
                    )
                    nc.tensor.matmul(
                        s[:, 512:1024],
                        qkv_sb[64:128, 1, t2 : t2 + 128],
                        qkv_sb[64:128, 0, t1 : t1 + 512],
                        start=True,
                        stop=True,
                    )

                def finalize_span(span):
                    u0, u1 = u_tiles.pop(span)
                    blk = (b * N + span * 512) // 512
                    for h, u in ((0, u0), (1, u1)):
                        zinv = zp.tile([1, 512], F32, tag=f"z{h}", name="zi")
                        nc.vector.reciprocal_approx_fast(zinv[0:1, :], u[0:1, :])
                        zb = zp.tile([65, 512], F32, tag=f"zb{h}", name="zb")
                        nc.gpsimd.partition_broadcast(
                            zb[0:65, :], zinv[0:1, :], channels=65
                        )
                        nrm = np_pool.tile([65, 512], BF16, tag=f"n{h}", name="nr")
                        nc.vector.tensor_tensor(
                            nrm[0:65, :], u[0:65, :], zb[0:65, :], op=ALU.mult
                        )
                        eng = nc.scalar if span == nspans - 1 else nc.sync
                        eng.dma_start(
                            out=a2a_in[
                                blk * 128 + h * 64 : blk * 128 + (h + 1) * 64, :
                            ],
                            in_=nrm[1:65, :],
                        )

                qk(0)
                for i in range(total):
                    span, ch = divmod(i, nchunks)
                    if i + 1 < total:
                        qk(i + 1)
                    if ch == 0:
                        u_tiles[span] = (
                            up.tile([128, 512], F32, tag="u0", name="u0"),
                            up.tile([128, 512], F32, tag="u1", name="u1"),
                        )
                    u0, u1 = u_tiles[span]
                    s = s_tiles.pop(i)
                    pt = ptp.tile([128, 1024], BF16, tag="pt", name="pt")
                    nc.scalar.activation(pt[:], s[:], AF.Exp, scale=SCALE)
                    gch = (b * N + ch * 128) // 128
                    nc.tensor.matmul(
                        u0[0:65, :],
                        vt1[:, gch, 0, :],
                        pt[:, 0:512],
                        start=(ch == 0),
                        stop=(ch == nchunks - 1),
                    )
                    nc.tensor.matmul(
                        u1[0:65, :],
                        vt1[:, gch, 1, :],
                        pt[:, 512:1024],
                        start=(ch == 0),
                        stop=(ch == nchunks - 1),
                    )
                    if extra is not None:
                        next(extra)
                    if ch == nchunks - 1:
                        finalize_span(span)

            # ---- phase 1: qkv(b0) bulk (c-outer, overlaps x DMA) ----
            with tc.tile_pool(name="qkv_psum", bufs=8, space="PSUM") as qp:
                qkv_bulk(qp, [0, 1], [0, 1, 2, 3])  # q,k batch 0
                qkv_bulk(qp, [2], [0, 1, 2, 3])  # v batch 0

            # ---- phase 2: attention(b0) w/ interleaved qkv(b1); then b1 ----
            with (
                tc.tile_pool(name="norm", bufs=4) as np_pool,
                tc.tile_pool(name="pt", bufs=3) as ptp,
                tc.tile_pool(name="zrow", bufs=4) as zp,
            ):
                with (
                    tc.tile_pool(name="s_psum", bufs=2, space="PSUM") as sp,
                    tc.tile_pool(name="u_psum", bufs=1, space="PSUM") as up,
                    tc.tile_pool(name="qi_psum", bufs=2, space="PSUM") as qip,
                ):
                    gen = qkv_interleaved_gen(qip, [0, 1, 2], [4, 5, 6, 7])
                    attention_batch(0, sp, up, np_pool, ptp, zp, extra=gen)
                    for _ in range(12 * 9 + 4):  # drain remaining b1 qkv
                        next(gen)
                with (
                    tc.tile_pool(name="s_psum2", bufs=2, space="PSUM") as sp2,
                    tc.tile_pool(name="u_psum2", bufs=2, space="PSUM") as up2,
                ):
                    attention_batch(1, sp2, up2, np_pool, ptp, zp, extra=None)

            # ---- phase 3: AllToAll (head-sharded -> token-sharded) ----
            nc.gpsimd.collective_compute(
                "AllToAll",
                ALU.bypass,
                replica_groups=[list(range(N_CORES))],
                ins=[a2a_in[:].opt()],
                outs=[a2a_out[:].opt()],
            )
            lo = a2a_out[0:512, :].rearrange("(g p) t -> p g t", p=128)
            hi = a2a_out[512:1024, :].rearrange("(g p) t -> p g t", p=128)
            nc.sync.dma_start(out=a2a_sb[:, 0:4, :], in_=lo)
            nc.scalar.dma_start(out=a2a_sb[:, 4:8, :], in_=hi)

            # ---- phase 4: proj (full weights, my 512 tokens) ----
            with tc.tile_pool(name="proj_psum", bufs=4, space="PSUM") as pp:
                for o in range(CCH):
                    ps = pp.tile([128, TPC], F32, tag="pps", name="pps")
                    for g in range(CCH):
                        nc.tensor.matmul(
                            ps[:],
                            pwt[:, g, o * 128 : (o + 1) * 128],
                            a2a_sb[:, g, :],
                            start=(g == 0),
                            stop=(g == CCH - 1),
                        )
                    nc.scalar.activation(
                        out_sb[:, o, :], ps[:], AF.Identity, bias=pb[:, o : o + 1]
                    )
                    nc.sync.dma_start(out=out_d[:, o, :], in_=out_sb[:, o, :])

    nc.compile()
    return nc


def shard_inputs(x, qkv_w, qkv_b, proj_w, proj_b):
    """Host-side sharding + layout prep. Returns in_maps for 8 cores."""
    bf = ml_dtypes.bfloat16
    x2 = np.ascontiguousarray(x.reshape(T, C).T)  # [C, T]
    xt = np.ascontiguousarray(x2.reshape(CCH, 128, T).transpose(1, 0, 2)).astype(bf)
    pwt_f = np.ascontiguousarray(proj_w.T)  # [j, o]
    pwt = np.ascontiguousarray(
        pwt_f.reshape(CCH, 128, C).transpose(1, 0, 2)
    ).astype(bf)
    pb = np.ascontiguousarray(proj_b.reshape(CCH, 128).T).astype(np.float32)

    in_maps = []
    for c in range(N_CORES):
        rows = lambda base: slice(base + 128 * c, base + 128 * (c + 1))
        wq = qkv_w[rows(0)]
        wk = qkv_w[rows(C)]
        wv = qkv_w[rows(2 * C)]
        wqkv = np.concatenate([wq, wk, wv], axis=0).T  # [C, 384]
        wqkv = np.ascontiguousarray(
            wqkv.reshape(CCH, 128, 384).transpose(1, 0, 2)
        ).astype(bf)
        bqkv = np.stack(
            [qkv_b[rows(0)], qkv_b[rows(C)], qkv_b[rows(2 * C)]], axis=1
        ).astype(np.float32)
        in_maps.append({"xt": xt, "wqkv": wqkv, "bqkv": bqkv, "pwt": pwt, "pb": pb})
    return in_maps


_CACHED_NC = None


def kernel(x, qkv_w, qkv_b, proj_w, proj_b, _trace=False, _tmpdir=None):
    global _CACHED_NC
    x = np.asarray(x, dtype=np.float32)
    qkv_w = np.asarray(qkv_w, dtype=np.float32)
    qkv_b = np.asarray(qkv_b, dtype=np.float32)
    proj_w = np.asarray(proj_w, dtype=np.float32)
    proj_b = np.asarray(proj_b, dtype=np.float32)

    in_maps = shard_inputs(x, qkv_w, qkv_b, proj_w, proj_b)
    if _CACHED_NC is None:
        _CACHED_NC = build_bass()
    res = run_bass_kernel_spmd(
        _CACHED_NC,
        in_maps,
        core_ids=list(range(N_CORES)),
        trace=_trace,
        tmpdir=_tmpdir,
    )
    # out per core: [128, CCH, TPC] f32, rows o = o_chunk*128 + p, cols = my toks
    shards = []
    for c in range(N_CORES):
        o = np.asarray(res.results[c]["out"], dtype=np.float32)
        ot = o.transpose(1, 0, 2).reshape(C, TPC)  # [1024 o, 512 t]
        shards.append(ot.T)  # [512 t, 1024 o]
    full = np.concatenate(shards, axis=0)  # [4096, 1024]
    out = full.reshape(B, N, C)
    if _trace:
        return out, res
    return out
